# revision 33
# baseline (speedup 1.0000x reference)
"""Adaptive temperature scaling loss on 8 TRN2 NeuronCores.

Data-parallel: B=65536 rows sharded 8 ways (8192 rows/core), C=1000.
Per core: 64 tiles of (128 rows x 1000). Heavy tensors fp16 in SBUF
(cast in-flight by SWDGE DMA); per-row stats f32.

Per row r: LTS = x.w_L ; H = sum p log p (via S1=sum x e^x, Z0=sum e^x);
T = clip(softplus(LTS + w_H*H/lnC + b), EPS); nll = (M - x_lbl)/T + ln Z2
with M = row max, Z2 = sum exp((x-M)/T). Mean nll all-reduced over cores.

Engine split per tile (measured): ACT exp1+Z0 / exp2+Z2 accum ~1.5us ea;
DVE: TT-max tree (~0.85us) + custom TENSOR_TENSOR_REDUCE for S1 and LTS
(~1.3us ea); GPSIMD: x_label indirect-DMA gather from DRAM + tile loads.

NOTE: codegen allows ONE semaphore wait per instruction; pool sizing,
op ordering, clock-priming and the vector-clock wait stripper below
keep every instruction at <=1 wait.
"""

import os
import sys
import types

import numpy as np

# The axon boot publishes its NTFF profile hook via `antenv.axon_hooks`;
# some images lack that module, which both disables tracing and crashes
# `run_bass_kernel_spmd(trace=True)`. Provide it before jax boots.
try:
    import antenv.axon_hooks  # noqa: F401
except ImportError:
    try:
        import antenv
        _hooks = types.ModuleType("antenv.axon_hooks")
        _hooks._hook = None

        def _set_hook(h):
            _hooks._hook = h

        def _get_hook():
            return _hooks._hook

        _hooks.set_axon_ntff_profile_hook = _set_hook
        _hooks.get_axon_ntff_profile_hook = _get_hook
        sys.modules["antenv.axon_hooks"] = _hooks
        antenv.axon_hooks = _hooks
        try:
            from trn_agent_boot.trn_boot import _ntff_profile_via_ctypes
            _hooks._hook = _ntff_profile_via_ctypes("/opt/axon/libaxon_pjrt.so")
        except Exception:
            pass
    except ImportError:
        pass

B, C = 65536, 1000
N_CORES = 8
ROWS = B // N_CORES          # 8192 rows per core
P = 128                      # partitions
NT = ROWS // P               # 64 tiles per core
GROUP = int(os.environ.get("AT_GROUP", "8"))
NG = NT // GROUP
TPB = int(os.environ.get("AT_TPB", "8"))  # tiles per DMA load block
XBUFS = int(os.environ.get("AT_XBUFS", "6"))
LOOKAHEAD = int(os.environ.get("AT_LA", "3"))
ROUTE = os.environ.get("AT_ROUTE", "26")
EPS = float(np.finfo(np.float32).eps)
LN_C = float(np.log(C))

_built = {}


def _build_nc():
    import concourse.bass as bass
    import concourse.tile as tile
    from concourse import mybir
    from contextlib import ExitStack

    f32 = mybir.dt.float32
    f16 = mybir.dt.float16
    i32 = mybir.dt.int32
    AF = mybir.ActivationFunctionType
    ALU = mybir.AluOpType

    nc = bass.Bass(num_devices=N_CORES)

    x_ext = nc.declare_dram_parameter("x", [ROWS, C], f32, isOutput=False)
    w_ext = nc.declare_dram_parameter("w_rep", [P, C], f32, isOutput=False)
    off_ext = nc.declare_dram_parameter("off", [P, NT], i32, isOutput=False)
    sc_ext = nc.declare_dram_parameter("sc", [P, 2], f32, isOutput=False)
    out_ext = nc.declare_dram_parameter("out", [1], f32, isOutput=True)

    cc_in = nc.dram_tensor("cc_in", [1], f32)
    cc_out = nc.dram_tensor("cc_out", [1], f32)

    # rows (TPB j + b)*128 + p  <->  x5[j][p, b, c]
    x5 = x_ext[:].rearrange("(j b p) c -> j p b c", b=TPB, p=P)
    xflat = x_ext[:].rearrange("r (c u) -> (r c) u", u=1)  # for the label gather

    with ExitStack() as ctx:
        tc = ctx.enter_context(tile.TileContext(nc))
        xpool = ctx.enter_context(tc.tile_pool(name="x", bufs=XBUFS))
        ppool = ctx.enter_context(tc.tile_pool(name="p", bufs=LOOKAHEAD * GROUP + 2))
        jdve = ctx.enter_context(tc.tile_pool(name="jdve", bufs=2))
        wppool = ctx.enter_context(tc.tile_pool(name="wp", bufs=3))
        jact = ctx.enter_context(tc.tile_pool(name="jact", bufs=2))
        treepool = ctx.enter_context(tc.tile_pool(name="tree", bufs=1))
        constpool = ctx.enter_context(tc.tile_pool(name="const", bufs=1))
        statpool = ctx.enter_context(tc.tile_pool(name="stat", bufs=NG))
        finpool = ctx.enter_context(tc.tile_pool(name="fin", bufs=2))
        tailpool = ctx.enter_context(tc.tile_pool(name="tail", bufs=1))

        # ---- first x block before everything (fastest compute start) ----
        xslot0 = xpool.tile([P, TPB, C], f16, tag="x", name="xs")
        for bb in range(TPB):
            nc.gpsimd.dma_start(out=xslot0[:, bb, :], in_=x5[0][:, bb, :])
        # ---- constants / metadata ----
        wt = constpool.tile([P, C], f16, tag="wt")
        nc.gpsimd.dma_start(out=wt[:], in_=w_ext[:])           # cast f32->f16
        offt = constpool.tile([P, NT], i32, tag="offt")
        nc.sync.dma_start(out=offt[:], in_=off_ext[:])
        sct = constpool.tile([P, 2], f32, tag="sct")
        nc.sync.dma_start(out=sct[:], in_=sc_ext[:])
        # prime gpsimd's clock on the offsets tile so later gathers only
        # wait on their own deps
        offdummy = constpool.tile([P, NT], i32, tag="offdummy")
        nc.gpsimd.tensor_copy(out=offdummy[:], in_=offt[:])
        # prime ACT's clock on DVE (and force the 0.0 bias const-AP's
        # memset to trace now), so exp1 instructions only wait their DMA
        actin = constpool.tile([1, 1], f32, tag="actin")
        nc.vector.memset(actin[:], 0.0)
        ones_t = constpool.tile([P, 1], f32, tag="ones")
        nc.vector.memset(ones_t[:], 1.0)
        # prime DVE's clock on the sct DMA so group-0 finals carry one wait
        sctdummy = constpool.tile([P, 2], f32, tag="sctdummy")
        nc.vector.tensor_copy(out=sctdummy[:], in_=sct[:])
        actout = constpool.tile([1, 1], f32, tag="actout")
        nc.scalar.activation(actout[:], actin[:], AF.Exp)

        gts = tailpool.tile([P, NG], f32, tag="gts")           # per-group row sums

        first_exp2 = {}          # g -> first exp2 instruction of group g
        for g in range(NG):
            Z0g = statpool.tile([P, GROUP], f32, tag="Z0", name="Z0")
            S1g = statpool.tile([P, GROUP], f32, tag="S1", name="S1")
            LTSg = statpool.tile([P, GROUP], f32, tag="LTS", name="LTS")
            Mg = statpool.tile([P, GROUP], f32, tag="M", name="M")
            XLg = statpool.tile([P, GROUP], f32, tag="XL", name="XL")
            Z2g = statpool.tile([P, GROUP], f32, tag="Z2", name="Z2")

            xtiles = []
            for jj in range(GROUP // TPB):
                j = g * (GROUP // TPB) + jj
                if g == 0 and jj == 0:
                    xslot = xslot0          # preloaded before the consts
                else:
                    xslot = xpool.tile([P, TPB, C], f16, tag="x", name="xs")
                    nc.gpsimd.dma_start(out=xslot[:], in_=x5[j])  # 2MB, cast
                # row max for the whole 4-tile slot via a 3D TT tree;
                # the first TT is the first DVE touch of the slot, so it
                # alone carries the DMA wait
                k0 = TPB * jj
                m1 = treepool.tile([P, TPB, 500], f16, tag="m1", name="m1")
                i_m1 = nc.vector.tensor_tensor(
                    out=m1[:], in0=xslot[:, :, 0:500],
                    in1=xslot[:, :, 500:1000], op=ALU.max)
                m2 = treepool.tile([P, TPB, 250], f16, tag="m2", name="m2")
                nc.vector.tensor_tensor(
                    out=m2[:], in0=m1[:, :, 0:250], in1=m1[:, :, 250:500],
                    op=ALU.max)
                m3 = treepool.tile([P, TPB, 125], f16, tag="m3", name="m3")
                nc.vector.tensor_tensor(
                    out=m3[:], in0=m2[:, :, 0:125], in1=m2[:, :, 125:250],
                    op=ALU.max)
                nc.vector.tensor_reduce(
                    out=Mg[:, k0:k0 + TPB], in_=m3[:],
                    axis=mybir.AxisListType.X, op=ALU.max)
                for bb in range(TPB):
                    k = TPB * jj + bb                          # idx within group
                    t = TPB * j + bb                           # global tile idx
                    xt = xslot[:, bb, :]
                    xtiles.append(xt)
                    # x_label gather straight from DRAM (f32 exact), one
                    # offset per partition
                    nc.gpsimd.indirect_dma_start(
                        out=XLg[:, k:k + 1], out_offset=None,
                        in_=xflat,
                        in_offset=bass.IndirectOffsetOnAxis(
                            ap=offt[:, t:t + 1], axis=0))
                    pt = ppool.tile([P, C], f16, tag="p", name="pt")
                    i_exp1 = nc.scalar.activation(pt[:], xt, AF.Exp,
                                         accum_out=Z0g[:, k:k + 1])
                    if k == 0 and g >= LOOKAHEAD:
                        # cap ACT lookahead so pt-slot reuse stays behind
                        # ACT's last DVE sync
                        tile.add_dep_helper(i_exp1.ins,
                                            first_exp2[g - LOOKAHEAD].ins,
                                            sync=False,
                                            reason="cap ACT exp1 lookahead")
                    # S1 = sum x*e^x  (one fused STT op, accum in f32)
                    junk = jdve.tile([P, C], f16, tag="junk", name="jd")
                    i_s1 = nc.vector.scalar_tensor_tensor(
                        out=junk[:], in0=xt, scalar=1.0, in1=pt[:],
                        op0=ALU.mult, op1=ALU.mult,
                        accum_out=S1g[:, k:k + 1])
                    if bb == 0:
                        tile.add_dep_helper(i_s1.ins, i_m1.ins, sync=False,
                                            reason="order DVE: max before S1")
                    # LTS = sum w*x; route 2 tiles/group through ACT
                    # (copy+accum) to balance the engines
                    if str(k) in ROUTE and g < NG - 1:
                        wprod = wppool.tile([P, C], f16, tag="wp", name="wp")
                        nc.vector.tensor_tensor(out=wprod[:], in0=xt,
                                                in1=wt[:], op=ALU.mult)
                        junka = jact.tile([P, C], f16, tag="junk", name="ja")
                        nc.scalar.activation(junka[:], wprod[:], AF.Copy,
                                             accum_out=LTSg[:, k:k + 1])
                    else:
                        junk = jdve.tile([P, C], f16, tag="junk", name="jd")
                        nc.vector.scalar_tensor_tensor(
                            out=junk[:], in0=xt, scalar=1.0, in1=wt[:],
                            op0=ALU.mult, op1=ALU.mult,
                            accum_out=LTSg[:, k:k + 1])

            # ---- per-row finals for this group (128 x GROUP f32) ----
            def ft(tag):
                return finpool.tile([P, GROUP], f32, tag=tag, name=tag)

            rZ0 = ft("rZ0"); nc.vector.reciprocal(rZ0[:], Z0g[:])
            epx = ft("epx"); nc.vector.tensor_tensor(out=epx[:], in0=S1g[:], in1=rZ0[:], op=ALU.mult)
            lZ0 = ft("lZ0"); nc.scalar.activation(lZ0[:], Z0g[:], AF.Ln)
            h = ft("h"); nc.vector.tensor_tensor(out=h[:], in0=epx[:], in1=lZ0[:], op=ALU.subtract)
            t2 = ft("t2"); nc.vector.scalar_tensor_tensor(out=t2[:], in0=h[:], scalar=sct[:, 0:1], in1=LTSg[:], op0=ALU.mult, op1=ALU.add)
            av = ft("av"); nc.vector.tensor_scalar(out=av[:], in0=t2[:], scalar1=sct[:, 1:2], scalar2=None, op0=ALU.add)
            # stable softplus: relu(a) + ln(1 + exp(-|a|)), then clip at EPS
            # |a| = max(-a, a) in one STT op; exp then ln(1+u) via Ln bias
            aa = ft("aa"); nc.vector.scalar_tensor_tensor(out=aa[:], in0=av[:], scalar=-1.0, in1=av[:], op0=ALU.mult, op1=ALU.max)
            en = ft("en"); nc.scalar.activation(en[:], aa[:], AF.Exp, scale=-1.0)
            l1 = ft("l1"); nc.scalar.activation(l1[:], en[:], AF.Ln, bias=1.0, scale=1.0)
            ra = ft("ra"); nc.vector.tensor_scalar(out=ra[:], in0=av[:], scalar1=0.0, scalar2=None, op0=ALU.max)
            Tv = ft("Tv"); nc.vector.tensor_tensor(out=Tv[:], in0=ra[:], in1=l1[:], op=ALU.add)
            Tc = ft("Tc"); nc.vector.tensor_scalar(out=Tc[:], in0=Tv[:], scalar1=EPS, scalar2=None, op0=ALU.max)
            invT = ft("invT"); nc.vector.reciprocal(invT[:], Tc[:])
            negms = ft("negms"); nc.vector.scalar_tensor_tensor(out=negms[:], in0=Mg[:], scalar=-1.0, in1=invT[:], op0=ALU.mult, op1=ALU.mult)

            # ---- pass 2: Z2 = sum exp((x - M)/T) per tile ----
            for k in range(GROUP):
                junk = jact.tile([P, C], f16, tag="junk", name="ja")
                i_exp2 = nc.scalar.activation(junk[:], xtiles[k], AF.Exp,
                                     bias=negms[:, k:k + 1],
                                     scale=invT[:, k:k + 1],
                                     accum_out=Z2g[:, k:k + 1])
                if k == 0:
                    first_exp2[g] = i_exp2

            lZ2 = ft("lZ2"); nc.scalar.activation(lZ2[:], Z2g[:], AF.Ln)
            d = ft("d"); nc.vector.tensor_tensor(out=d[:], in0=Mg[:], in1=XLg[:], op=ALU.subtract)
            z = ft("z"); nc.vector.tensor_tensor(out=z[:], in0=d[:], in1=invT[:], op=ALU.mult)
            rn = ft("rn"); nc.vector.tensor_tensor(out=rn[:], in0=z[:], in1=lZ2[:], op=ALU.add)
            junkf = finpool.tile([P, GROUP], f32, tag="junkf", name="junkf")
            nc.vector.tensor_scalar(out=junkf[:], in0=rn[:], scalar1=1.0,
                                    scalar2=None, op0=ALU.mult, op1=ALU.add,
                                    accum_out=gts[:, g:g + 1])

        # ---- tail: sum over rows (DVE), partitions (PE), cores (CC) ----
        rowtot = tailpool.tile([P, 1], f32, tag="rowtot")
        junkg = tailpool.tile([P, NG], f32, tag="junkg")
        nc.vector.tensor_scalar(out=junkg[:], in0=gts[:], scalar1=1.0,
                                scalar2=None, op0=ALU.mult, op1=ALU.add,
                                accum_out=rowtot[:])
        pspool = ctx.enter_context(tc.tile_pool(name="ps", bufs=1,
                                                space="PSUM"))
        acc_ps = pspool.tile([1, 1], f32, tag="accps")
        nc.tensor.matmul(acc_ps[:], lhsT=rowtot[:], rhs=ones_t[:],
                         start=True, stop=True)
        part = tailpool.tile([1, 1], f32, tag="part")
        nc.vector.tensor_scalar(out=part[:], in0=acc_ps[:], scalar1=1.0 / B,
                                scalar2=None, op0=ALU.mult)
        nc.sync.dma_start(out=cc_in[:], in_=part[:])
        nc.gpsimd.collective_compute(
            "AllReduce", ALU.add,
            replica_groups=[list(range(N_CORES))],
            ins=[cc_in[:]], outs=[cc_out[:]])
        nc.sync.dma_start(out=out_ext[:], in_=cc_out[:])

    _strip_self_waits(nc)
    return nc


def _strip_self_waits(nc):
    """Drop semaphore waits that are already implied — by same-engine
    program order or transitively through other waits (vector clocks).
    Codegen allows only one hardware wait slot per instruction, and
    Tile sometimes emits implied extras (e.g. a WAR wait on a pool slot
    whose release is already ordered through another engine's sync).

    Assumes FIFO retirement per engine and per DMA-semaphore lane (the
    same assumptions Tile's cumulative thresholds rely on)."""
    from concourse import mybir

    eng_clock = {}            # engine -> {sem: tick}
    sem_hist = {}             # sem -> list of (tick_value, clock_dict)

    def clock_at(sem, thr):
        hist = sem_hist.get(sem)
        if not hist:
            return {}
        out = {}
        for tick, clk in hist:
            for kk, v in clk.items():
                if v > out.get(kk, -1):
                    out[kk] = v
            if tick >= thr:
                break
        return out

    for blk in nc.m.functions[0].blocks:
        for inst in blk.instructions:
            eng = str(inst.engine)
            cur = dict(eng_clock.get(eng, {}))
            si = inst.sync_info
            waits = list(si.on_wait) if si is not None and si.on_wait else []
            wclocks = [clock_at(w.ant_name, w.wait_value) for w in waits]
            if len(waits) >= 2:
                kept = []
                kept_idx = []
                for i, w in enumerate(waits):
                    obs = dict(cur)
                    # only waits we keep, or haven't decided yet, count
                    others = kept_idx + list(range(i + 1, len(waits)))
                    for j in others:
                        for kk, v in wclocks[j].items():
                            if v > obs.get(kk, -1):
                                obs[kk] = v
                    if obs.get(w.ant_name, -1) >= w.wait_value:
                        continue          # implied by the others
                    kept.append(w)
                    kept_idx.append(i)
                if len(kept) != len(waits):
                    inst.sync_info = mybir.SyncInfo(on_wait=kept,
                                                    on_update=si.on_update)
                    waits = kept
                    wclocks = [clock_at(w.ant_name, w.wait_value)
                               for w in waits]
            # advance this engine's clock
            for i, w in enumerate(waits):
                for kk, v in wclocks[i].items():
                    if v > cur.get(kk, -1):
                        cur[kk] = v
                if w.wait_value > cur.get(w.ant_name, -1):
                    cur[w.ant_name] = w.wait_value
            ups = si.on_update if si is not None and si.on_update else []
            for u in ups:
                sem = u.ant_name
                hist = sem_hist.setdefault(sem, [])
                prev = hist[-1][0] if hist else 0
                newtick = prev + (u.update_value or 1)
                cc = dict(cur)
                cc[sem] = newtick
                hist.append((newtick, cc))
                cur[sem] = newtick
            eng_clock[eng] = cur


def _prep_inputs(Simple_vector, label_list, w_L, w_H, b):
    x = np.ascontiguousarray(np.asarray(Simple_vector, dtype=np.float32))
    lbl = np.asarray(label_list).astype(np.int64)
    w_L = np.asarray(w_L, dtype=np.float32)
    sc = np.empty((P, 2), dtype=np.float32)
    sc[:, 0] = np.float32(np.asarray(w_H, dtype=np.float32)[0] / np.float32(LN_C))
    sc[:, 1] = np.float32(np.asarray(b, dtype=np.float32)[0])
    w_rep = np.ascontiguousarray(np.broadcast_to(w_L[None, :], (P, C)))
    in_maps = []
    for cid in range(N_CORES):
        r0 = cid * ROWS
        shard = x[r0:r0 + ROWS]
        lbl_shard = lbl[r0:r0 + ROWS]
        # off[p, t] = flat element index of (row 128 t + p, its label)
        rows_local = np.arange(ROWS, dtype=np.int64)
        flat = rows_local * C + lbl_shard
        off = np.ascontiguousarray(
            flat.reshape(NT, P).T.astype(np.int32))    # [p, t]
        in_maps.append({
            "x": shard,
            "w_rep": w_rep,
            "off": off,
            "sc": sc,
            "out": np.zeros((1,), dtype=np.float32),
        })
    return in_maps


def kernel(Simple_vector, label_list, w_L, w_H, b):
    from concourse.bass_utils import run_bass_kernel_spmd

    key = "nc"
    if key not in _built:
        _built[key] = _build_nc()
    nc = _built[key]

    in_maps = _prep_inputs(Simple_vector, label_list, w_L, w_H, b)
    res = run_bass_kernel_spmd(nc, in_maps, core_ids=list(range(N_CORES)))
    _built["last_result"] = res
    if res.exec_time_ns is not None:
        print(f"HW exec time: {res.exec_time_ns} ns")
        itp = res.instructions_and_trace
        if itp is not None:
            print(f"trace: {itp[1]}")
    out = np.asarray(res.results[0]["out"]).reshape(())
    return out.astype(np.float32)


if __name__ == "__main__":
    rng = np.random.default_rng(0)
    xs = rng.standard_normal((B, C), dtype=np.float32)
    ls = rng.integers(0, C, size=(B,)).astype(np.int32)
    wl = rng.standard_normal((C,), dtype=np.float32)
    wh = np.ones((1,), np.float32)
    bb = np.ones((1,), np.float32)
    print(kernel(xs, ls, wl, wh, bb))


# revision 34
# speedup vs baseline: 1.2077x; 1.2077x over previous
"""Adaptive temperature scaling loss on 8 TRN2 NeuronCores.

Data-parallel: B=65536 rows sharded 8 ways (8192 rows/core), C=1000.
Per core: 64 tiles of (128 rows x 1000). Heavy tensors fp16 in SBUF
(cast in-flight by SWDGE DMA); per-row stats f32.

Per row r: LTS = x.w_L ; H = sum p log p (via S1=sum x e^x, Z0=sum e^x);
T = clip(softplus(LTS + w_H*H/lnC + b), EPS); nll = (M - x_lbl)/T + ln Z2
with M = row max, Z2 = sum exp((x-M)/T). Mean nll all-reduced over cores.

Engine split per tile (measured): ACT exp1+Z0 / exp2+Z2 accum ~1.5us ea;
DVE: TT-max tree (~0.85us) + custom TENSOR_TENSOR_REDUCE for S1 and LTS
(~1.3us ea); GPSIMD: x_label indirect-DMA gather from DRAM + tile loads.

NOTE: codegen allows ONE semaphore wait per instruction; pool sizing,
op ordering, clock-priming and the vector-clock wait stripper below
keep every instruction at <=1 wait.
"""

import os
import sys
import types

import numpy as np

# The axon boot publishes its NTFF profile hook via `antenv.axon_hooks`;
# some images lack that module, which both disables tracing and crashes
# `run_bass_kernel_spmd(trace=True)`. Provide it before jax boots.
try:
    import antenv.axon_hooks  # noqa: F401
except ImportError:
    try:
        import antenv
        _hooks = types.ModuleType("antenv.axon_hooks")
        _hooks._hook = None

        def _set_hook(h):
            _hooks._hook = h

        def _get_hook():
            return _hooks._hook

        _hooks.set_axon_ntff_profile_hook = _set_hook
        _hooks.get_axon_ntff_profile_hook = _get_hook
        sys.modules["antenv.axon_hooks"] = _hooks
        antenv.axon_hooks = _hooks
        try:
            from trn_agent_boot.trn_boot import _ntff_profile_via_ctypes
            _hooks._hook = _ntff_profile_via_ctypes("/opt/axon/libaxon_pjrt.so")
        except Exception:
            pass
    except ImportError:
        pass

B, C = 65536, 1000
N_CORES = 8
ROWS = B // N_CORES          # 8192 rows per core
P = 128                      # partitions
NT = ROWS // P               # 64 tiles per core
GROUP = int(os.environ.get("AT_GROUP", "8"))
NG = NT // GROUP
TPB = int(os.environ.get("AT_TPB", "8"))  # tiles per DMA load block
XBUFS = int(os.environ.get("AT_XBUFS", "6"))
LOOKAHEAD = int(os.environ.get("AT_LA", "3"))
ROUTE = os.environ.get("AT_ROUTE", "26")
EPS = float(np.finfo(np.float32).eps)
LN_C = float(np.log(C))

_built = {}


def _build_nc():
    import concourse.bass as bass
    import concourse.tile as tile
    from concourse import mybir
    from contextlib import ExitStack

    f32 = mybir.dt.float32
    f16 = mybir.dt.float16
    i32 = mybir.dt.int32
    AF = mybir.ActivationFunctionType
    ALU = mybir.AluOpType

    nc = bass.Bass(num_devices=N_CORES)

    x_ext = nc.declare_dram_parameter("x", [ROWS, C], f32, isOutput=False)
    w_ext = nc.declare_dram_parameter("w_rep", [P, C], f32, isOutput=False)
    off_ext = nc.declare_dram_parameter("off", [P, NT], i32, isOutput=False)
    sc_ext = nc.declare_dram_parameter("sc", [P, 2], f32, isOutput=False)
    out_ext = nc.declare_dram_parameter("out", [1], f32, isOutput=True)

    cc_in = nc.dram_tensor("cc_in", [1], f32)
    cc_out = nc.dram_tensor("cc_out", [1], f32)

    # rows (TPB j + b)*128 + p  <->  x5[j][p, b, c]
    x5 = x_ext[:].rearrange("(j b p) c -> j p b c", b=TPB, p=P)
    xflat = x_ext[:].rearrange("r (c u) -> (r c) u", u=1)  # for the label gather

    with ExitStack() as ctx:
        tc = ctx.enter_context(tile.TileContext(nc))
        xpool = ctx.enter_context(tc.tile_pool(name="x", bufs=XBUFS))
        ppool = ctx.enter_context(tc.tile_pool(name="p", bufs=LOOKAHEAD * GROUP + 2))
        jdve = ctx.enter_context(tc.tile_pool(name="jdve", bufs=2))
        wppool = ctx.enter_context(tc.tile_pool(name="wp", bufs=3))
        jact = ctx.enter_context(tc.tile_pool(name="jact", bufs=2))
        treepool = ctx.enter_context(tc.tile_pool(name="tree", bufs=1))
        constpool = ctx.enter_context(tc.tile_pool(name="const", bufs=1))
        statpool = ctx.enter_context(tc.tile_pool(name="stat", bufs=NG))
        finpool = ctx.enter_context(tc.tile_pool(name="fin", bufs=2))
        tailpool = ctx.enter_context(tc.tile_pool(name="tail", bufs=1))

        # ---- first x block before everything (fastest compute start) ----
        xslot0 = xpool.tile([P, TPB, C], f16, tag="x", name="xs")
        for bb in range(TPB):
            nc.gpsimd.dma_start(out=xslot0[:, bb, :], in_=x5[0][:, bb, :])
        # ---- constants / metadata ----
        wt = constpool.tile([P, C], f16, tag="wt")
        nc.gpsimd.dma_start(out=wt[:], in_=w_ext[:])           # cast f32->f16
        offt = constpool.tile([P, NT], i32, tag="offt")
        nc.sync.dma_start(out=offt[:], in_=off_ext[:])
        sct = constpool.tile([P, 2], f32, tag="sct")
        nc.sync.dma_start(out=sct[:], in_=sc_ext[:])
        # prime gpsimd's clock on the offsets tile so later gathers only
        # wait on their own deps
        offdummy = constpool.tile([P, NT], i32, tag="offdummy")
        nc.gpsimd.tensor_copy(out=offdummy[:], in_=offt[:])
        # prime ACT's clock on DVE (and force the 0.0 bias const-AP's
        # memset to trace now), so exp1 instructions only wait their DMA
        actin = constpool.tile([1, 1], f32, tag="actin")
        nc.vector.memset(actin[:], 0.0)
        ones_t = constpool.tile([P, 1], f32, tag="ones")
        nc.vector.memset(ones_t[:], 1.0)
        # prime DVE's clock on the sct DMA so group-0 finals carry one wait
        sctdummy = constpool.tile([P, 2], f32, tag="sctdummy")
        nc.vector.tensor_copy(out=sctdummy[:], in_=sct[:])
        actout = constpool.tile([1, 1], f32, tag="actout")
        nc.scalar.activation(actout[:], actin[:], AF.Exp)

        gts = tailpool.tile([P, NG], f32, tag="gts")           # per-group row sums

        # slot loads are issued one group ahead of the per-tile gathers so
        # the (slow, Q7-serial) indirect gathers never delay a data load
        slot_handles = {0: xslot0}

        def issue_load(jj):
            sl = xpool.tile([P, TPB, C], f16, tag="x", name="xs")
            nc.gpsimd.dma_start(out=sl[:], in_=x5[jj])         # 4MB read, cast
            slot_handles[jj] = sl

        nblocks = NT // TPB
        if nblocks > 1:
            issue_load(1)

        first_exp2 = {}          # g -> first exp2 instruction of group g
        for g in range(NG):
            Z0g = statpool.tile([P, GROUP], f32, tag="Z0", name="Z0")
            S1g = statpool.tile([P, GROUP], f32, tag="S1", name="S1")
            LTSg = statpool.tile([P, GROUP], f32, tag="LTS", name="LTS")
            Mg = statpool.tile([P, GROUP], f32, tag="M", name="M")
            XLg = statpool.tile([P, GROUP], f32, tag="XL", name="XL")
            Z2g = statpool.tile([P, GROUP], f32, tag="Z2", name="Z2")

            xtiles = []
            for jj in range(GROUP // TPB):
                j = g * (GROUP // TPB) + jj
                if j + 1 < nblocks and (j + 1) not in slot_handles:
                    issue_load(j + 1)
                xslot = slot_handles.pop(j)
                # row max for the whole 4-tile slot via a 3D TT tree;
                # the first TT is the first DVE touch of the slot, so it
                # alone carries the DMA wait
                k0 = TPB * jj
                m1 = treepool.tile([P, TPB, 500], f16, tag="m1", name="m1")
                i_m1 = nc.vector.tensor_tensor(
                    out=m1[:], in0=xslot[:, :, 0:500],
                    in1=xslot[:, :, 500:1000], op=ALU.max)
                m2 = treepool.tile([P, TPB, 250], f16, tag="m2", name="m2")
                nc.vector.tensor_tensor(
                    out=m2[:], in0=m1[:, :, 0:250], in1=m1[:, :, 250:500],
                    op=ALU.max)
                m3 = treepool.tile([P, TPB, 125], f16, tag="m3", name="m3")
                nc.vector.tensor_tensor(
                    out=m3[:], in0=m2[:, :, 0:125], in1=m2[:, :, 125:250],
                    op=ALU.max)
                nc.vector.tensor_reduce(
                    out=Mg[:, k0:k0 + TPB], in_=m3[:],
                    axis=mybir.AxisListType.X, op=ALU.max)
                for bb in range(TPB):
                    k = TPB * jj + bb                          # idx within group
                    t = TPB * j + bb                           # global tile idx
                    xt = xslot[:, bb, :]
                    xtiles.append(xt)
                    # x_label gather straight from DRAM (f32 exact), one
                    # offset per partition
                    nc.gpsimd.indirect_dma_start(
                        out=XLg[:, k:k + 1], out_offset=None,
                        in_=xflat,
                        in_offset=bass.IndirectOffsetOnAxis(
                            ap=offt[:, t:t + 1], axis=0))
                    pt = ppool.tile([P, C], f16, tag="p", name="pt")
                    i_exp1 = nc.scalar.activation(pt[:], xt, AF.Exp,
                                         accum_out=Z0g[:, k:k + 1])
                    if k == 0 and g >= LOOKAHEAD:
                        # cap ACT lookahead so pt-slot reuse stays behind
                        # ACT's last DVE sync
                        tile.add_dep_helper(i_exp1.ins,
                                            first_exp2[g - LOOKAHEAD].ins,
                                            sync=False,
                                            reason="cap ACT exp1 lookahead")
                    # S1 = sum x*e^x  (one fused STT op, accum in f32)
                    junk = jdve.tile([P, C], f16, tag="junk", name="jd")
                    i_s1 = nc.vector.scalar_tensor_tensor(
                        out=junk[:], in0=xt, scalar=1.0, in1=pt[:],
                        op0=ALU.mult, op1=ALU.mult,
                        accum_out=S1g[:, k:k + 1])
                    if bb == 0:
                        tile.add_dep_helper(i_s1.ins, i_m1.ins, sync=False,
                                            reason="order DVE: max before S1")
                    # LTS = sum w*x; route 2 tiles/group through ACT
                    # (copy+accum) to balance the engines
                    if str(k) in ROUTE and g < NG - 1:
                        wprod = wppool.tile([P, C], f16, tag="wp", name="wp")
                        nc.vector.tensor_tensor(out=wprod[:], in0=xt,
                                                in1=wt[:], op=ALU.mult)
                        junka = jact.tile([P, C], f16, tag="junk", name="ja")
                        nc.scalar.activation(junka[:], wprod[:], AF.Copy,
                                             accum_out=LTSg[:, k:k + 1])
                    else:
                        junk = jdve.tile([P, C], f16, tag="junk", name="jd")
                        nc.vector.scalar_tensor_tensor(
                            out=junk[:], in0=xt, scalar=1.0, in1=wt[:],
                            op0=ALU.mult, op1=ALU.mult,
                            accum_out=LTSg[:, k:k + 1])

            # ---- per-row finals for this group (128 x GROUP f32) ----
            def ft(tag):
                return finpool.tile([P, GROUP], f32, tag=tag, name=tag)

            rZ0 = ft("rZ0"); nc.vector.reciprocal(rZ0[:], Z0g[:])
            epx = ft("epx"); nc.vector.tensor_tensor(out=epx[:], in0=S1g[:], in1=rZ0[:], op=ALU.mult)
            lZ0 = ft("lZ0"); nc.scalar.activation(lZ0[:], Z0g[:], AF.Ln)
            h = ft("h"); nc.vector.tensor_tensor(out=h[:], in0=epx[:], in1=lZ0[:], op=ALU.subtract)
            t2 = ft("t2"); nc.vector.scalar_tensor_tensor(out=t2[:], in0=h[:], scalar=sct[:, 0:1], in1=LTSg[:], op0=ALU.mult, op1=ALU.add)
            av = ft("av"); nc.vector.tensor_scalar(out=av[:], in0=t2[:], scalar1=sct[:, 1:2], scalar2=None, op0=ALU.add)
            # stable softplus: relu(a) + ln(1 + exp(-|a|)), then clip at EPS
            # |a| = max(-a, a) in one STT op; exp then ln(1+u) via Ln bias
            aa = ft("aa"); nc.vector.scalar_tensor_tensor(out=aa[:], in0=av[:], scalar=-1.0, in1=av[:], op0=ALU.mult, op1=ALU.max)
            en = ft("en"); nc.scalar.activation(en[:], aa[:], AF.Exp, scale=-1.0)
            l1 = ft("l1"); nc.scalar.activation(l1[:], en[:], AF.Ln, bias=1.0, scale=1.0)
            ra = ft("ra"); nc.vector.tensor_scalar(out=ra[:], in0=av[:], scalar1=0.0, scalar2=None, op0=ALU.max)
            Tv = ft("Tv"); nc.vector.tensor_tensor(out=Tv[:], in0=ra[:], in1=l1[:], op=ALU.add)
            Tc = ft("Tc"); nc.vector.tensor_scalar(out=Tc[:], in0=Tv[:], scalar1=EPS, scalar2=None, op0=ALU.max)
            invT = ft("invT"); nc.vector.reciprocal(invT[:], Tc[:])
            negms = ft("negms"); nc.vector.scalar_tensor_tensor(out=negms[:], in0=Mg[:], scalar=-1.0, in1=invT[:], op0=ALU.mult, op1=ALU.mult)

            # ---- pass 2: Z2 = sum exp((x - M)/T) per tile ----
            for k in range(GROUP):
                junk = jact.tile([P, C], f16, tag="junk", name="ja")
                i_exp2 = nc.scalar.activation(junk[:], xtiles[k], AF.Exp,
                                     bias=negms[:, k:k + 1],
                                     scale=invT[:, k:k + 1],
                                     accum_out=Z2g[:, k:k + 1])
                if k == 0:
                    first_exp2[g] = i_exp2

            lZ2 = ft("lZ2"); nc.scalar.activation(lZ2[:], Z2g[:], AF.Ln)
            d = ft("d"); nc.vector.tensor_tensor(out=d[:], in0=Mg[:], in1=XLg[:], op=ALU.subtract)
            z = ft("z"); nc.vector.tensor_tensor(out=z[:], in0=d[:], in1=invT[:], op=ALU.mult)
            rn = ft("rn"); nc.vector.tensor_tensor(out=rn[:], in0=z[:], in1=lZ2[:], op=ALU.add)
            junkf = finpool.tile([P, GROUP], f32, tag="junkf", name="junkf")
            nc.vector.tensor_scalar(out=junkf[:], in0=rn[:], scalar1=1.0,
                                    scalar2=None, op0=ALU.mult, op1=ALU.add,
                                    accum_out=gts[:, g:g + 1])

        # ---- tail: sum over rows (DVE), partitions (PE), cores (CC) ----
        rowtot = tailpool.tile([P, 1], f32, tag="rowtot")
        junkg = tailpool.tile([P, NG], f32, tag="junkg")
        nc.vector.tensor_scalar(out=junkg[:], in0=gts[:], scalar1=1.0,
                                scalar2=None, op0=ALU.mult, op1=ALU.add,
                                accum_out=rowtot[:])
        pspool = ctx.enter_context(tc.tile_pool(name="ps", bufs=1,
                                                space="PSUM"))
        acc_ps = pspool.tile([1, 1], f32, tag="accps")
        nc.tensor.matmul(acc_ps[:], lhsT=rowtot[:], rhs=ones_t[:],
                         start=True, stop=True)
        part = tailpool.tile([1, 1], f32, tag="part")
        nc.vector.tensor_scalar(out=part[:], in0=acc_ps[:], scalar1=1.0 / B,
                                scalar2=None, op0=ALU.mult)
        nc.sync.dma_start(out=cc_in[:], in_=part[:])
        nc.gpsimd.collective_compute(
            "AllReduce", ALU.add,
            replica_groups=[list(range(N_CORES))],
            ins=[cc_in[:]], outs=[cc_out[:]])
        nc.sync.dma_start(out=out_ext[:], in_=cc_out[:])

    _strip_self_waits(nc)
    return nc


def _strip_self_waits(nc):
    """Drop semaphore waits that are already implied — by same-engine
    program order or transitively through other waits (vector clocks).
    Codegen allows only one hardware wait slot per instruction, and
    Tile sometimes emits implied extras (e.g. a WAR wait on a pool slot
    whose release is already ordered through another engine's sync).

    Assumes FIFO retirement per engine and per DMA-semaphore lane (the
    same assumptions Tile's cumulative thresholds rely on)."""
    from concourse import mybir

    eng_clock = {}            # engine -> {sem: tick}
    sem_hist = {}             # sem -> list of (tick_value, clock_dict)

    def clock_at(sem, thr):
        hist = sem_hist.get(sem)
        if not hist:
            return {}
        out = {}
        for tick, clk in hist:
            for kk, v in clk.items():
                if v > out.get(kk, -1):
                    out[kk] = v
            if tick >= thr:
                break
        return out

    for blk in nc.m.functions[0].blocks:
        for inst in blk.instructions:
            eng = str(inst.engine)
            cur = dict(eng_clock.get(eng, {}))
            si = inst.sync_info
            waits = list(si.on_wait) if si is not None and si.on_wait else []
            wclocks = [clock_at(w.ant_name, w.wait_value) for w in waits]
            if len(waits) >= 2:
                kept = []
                kept_idx = []
                for i, w in enumerate(waits):
                    obs = dict(cur)
                    # only waits we keep, or haven't decided yet, count
                    others = kept_idx + list(range(i + 1, len(waits)))
                    for j in others:
                        for kk, v in wclocks[j].items():
                            if v > obs.get(kk, -1):
                                obs[kk] = v
                    if obs.get(w.ant_name, -1) >= w.wait_value:
                        continue          # implied by the others
                    kept.append(w)
                    kept_idx.append(i)
                if len(kept) != len(waits):
                    inst.sync_info = mybir.SyncInfo(on_wait=kept,
                                                    on_update=si.on_update)
                    waits = kept
                    wclocks = [clock_at(w.ant_name, w.wait_value)
                               for w in waits]
            # advance this engine's clock
            for i, w in enumerate(waits):
                for kk, v in wclocks[i].items():
                    if v > cur.get(kk, -1):
                        cur[kk] = v
                if w.wait_value > cur.get(w.ant_name, -1):
                    cur[w.ant_name] = w.wait_value
            ups = si.on_update if si is not None and si.on_update else []
            for u in ups:
                sem = u.ant_name
                hist = sem_hist.setdefault(sem, [])
                prev = hist[-1][0] if hist else 0
                newtick = prev + (u.update_value or 1)
                cc = dict(cur)
                cc[sem] = newtick
                hist.append((newtick, cc))
                cur[sem] = newtick
            eng_clock[eng] = cur


def _prep_inputs(Simple_vector, label_list, w_L, w_H, b):
    x = np.ascontiguousarray(np.asarray(Simple_vector, dtype=np.float32))
    lbl = np.asarray(label_list).astype(np.int64)
    w_L = np.asarray(w_L, dtype=np.float32)
    sc = np.empty((P, 2), dtype=np.float32)
    sc[:, 0] = np.float32(np.asarray(w_H, dtype=np.float32)[0] / np.float32(LN_C))
    sc[:, 1] = np.float32(np.asarray(b, dtype=np.float32)[0])
    w_rep = np.ascontiguousarray(np.broadcast_to(w_L[None, :], (P, C)))
    in_maps = []
    for cid in range(N_CORES):
        r0 = cid * ROWS
        shard = x[r0:r0 + ROWS]
        lbl_shard = lbl[r0:r0 + ROWS]
        # off[p, t] = flat element index of (row 128 t + p, its label)
        rows_local = np.arange(ROWS, dtype=np.int64)
        flat = rows_local * C + lbl_shard
        off = np.ascontiguousarray(
            flat.reshape(NT, P).T.astype(np.int32))    # [p, t]
        in_maps.append({
            "x": shard,
            "w_rep": w_rep,
            "off": off,
            "sc": sc,
            "out": np.zeros((1,), dtype=np.float32),
        })
    return in_maps


def kernel(Simple_vector, label_list, w_L, w_H, b):
    from concourse.bass_utils import run_bass_kernel_spmd

    key = "nc"
    if key not in _built:
        _built[key] = _build_nc()
    nc = _built[key]

    in_maps = _prep_inputs(Simple_vector, label_list, w_L, w_H, b)
    res = run_bass_kernel_spmd(nc, in_maps, core_ids=list(range(N_CORES)))
    _built["last_result"] = res
    if res.exec_time_ns is not None:
        print(f"HW exec time: {res.exec_time_ns} ns")
        itp = res.instructions_and_trace
        if itp is not None:
            print(f"trace: {itp[1]}")
    out = np.asarray(res.results[0]["out"]).reshape(())
    return out.astype(np.float32)


if __name__ == "__main__":
    rng = np.random.default_rng(0)
    xs = rng.standard_normal((B, C), dtype=np.float32)
    ls = rng.integers(0, C, size=(B,)).astype(np.int32)
    wl = rng.standard_normal((C,), dtype=np.float32)
    wh = np.ones((1,), np.float32)
    bb = np.ones((1,), np.float32)
    print(kernel(xs, ls, wl, wh, bb))
